# revision 1
# baseline (speedup 1.0000x reference)
"""CBTree bottom-up fold kernel for 8 trn2 NeuronCores.

Problem: complete 4-ary tree, 9 levels, 87381 nodes in BFS order, d=256.
  leaves (level 8): h = vectors[21845:]
  internal node:    h = tanh(sum_i W_i @ h_child_i + vectors[node])
  where W_i = lc[i]*Wl + rc[i]*Wr,  lc=[1,2/3,1/3,0], rc=[0,1/3,2/3,1].

Strategy (data-parallel over sibling groups):
  - Shard every level contiguously over 8 cores. Children of a core's
    parents are exactly the core's own previous-level outputs, so
    levels 7..4 run with zero communication.
  - One 32KB AllGather of the level-4 states (256 nodes), then every
    core redundantly folds levels 3..0 and writes the root.
  - On chip h lives transposed ([d, nodes], d split into two 128-row
    partition halves) so the tensor engine contracts over d. The
    host-side sharding step hands each core its slices already in this
    [d, nodes] layout (a zero-FLOP relayout done while slicing), so the
    device spends no PE/DVE cycles on transposes: level l is 8
    accumulating 128x128xN matmuls per output half (4 sibling
    positions x 2 d-halves), a 9th identity-weight matmul that adds
    the bias vector in PSUM, and a tanh on the scalar engine writing
    the next level's tile directly.
  - Default dtype is fp16 (fp32 PSUM accumulation): vs fp32r it halves
    every DMA stream and runs full-rate at any moving dim (fp32r drops
    to 1/4 rate below N=256), for ~3.4e-3 scale-relative error vs the
    fp32 reference (fp32r fallback: _mode="f32r", 1.1e-3, ~30% slower).
  - The root would be an N=1 matmul (invalid ISA), so the last level
    computes 4 replicated copies of the root.
"""

import numpy as np

F32 = None  # set on first _lazy_imports()

_BASS = {}


def _lazy_imports():
    global bass, bacc, mybir, tile, make_identity, run_bass_kernel_spmd, F32
    import concourse.bass as bass
    import concourse.mybir as mybir
    from concourse import bacc
    import concourse.tile as tile
    from concourse.masks import make_identity
    from concourse.bass_utils import run_bass_kernel_spmd
    F32 = mybir.dt.float32


N_CORES = 8
D = 256
B = 4
L = 9
SIZES = [B**l for l in range(L)]            # [1,4,16,64,256,1024,4096,16384,65536]
OFFSETS = np.concatenate([[0], np.cumsum(SIZES)])  # [0,1,5,21,85,341,1365,5461,21845,87381]
N_LEAF_CORE = SIZES[8] // N_CORES           # 8192
# local (sharded) levels produce parents at levels 7..4
LOC_LEVELS = [7, 6, 5, 4, 3]
LOC_PAR = {l: SIZES[l] // N_CORES for l in LOC_LEVELS}   # 2048,512,128,32,8
N_VECS_LOC = sum(LOC_PAR.values())          # 2720
N_VECS_TAIL = int(OFFSETS[4]) + 4           # 85 real rows + 4x replicated root row


def _build_nc(mode="fp16", reps=1, probe=None, WARM0=0, WARMC=0, leaf16=False, CHUNKS=None, CSZ=1024, BIAS_DVE=()):
    if CHUNKS is None:
        CHUNKS = {7: 256, 6: 128, 5: 64, 4: 16}
    key = ("nc", mode, reps, probe, WARM0, WARMC, leaf16, tuple(sorted(CHUNKS.items())), CSZ, tuple(BIAS_DVE))
    if key in _BASS:
        return _BASS[key]
    nc = bacc.Bacc(num_devices=N_CORES)
    mmdt = {"f32r": mybir.dt.float32r, "fp32": F32,
            "fp16": mybir.dt.float16}[mode]
    bf16 = mybir.dt.bfloat16
    leafdt = bf16 if leaf16 else mmdt
    dsz = 2 if mode == "fp16" else 4

    # all h/vec tensors arrive transposed: [256 = 2x128 d-rows, n nodes]
    leavesT = nc.declare_dram_parameter("leavesT", [D, N_LEAF_CORE], leafdt, isOutput=False)
    if leaf16:
        wmat16 = nc.declare_dram_parameter("wmat16", [128, 17 * 128], bf16, isOutput=False)
    vecs_locT = nc.declare_dram_parameter("vecs_locT", [D, N_VECS_LOC], mmdt, isOutput=False)
    vecs_tailT = nc.declare_dram_parameter("vecs_tailT", [D, N_VECS_TAIL], mmdt, isOutput=False)
    wmat = nc.declare_dram_parameter("wmat", [128, 17 * 128], mmdt, isOutput=False)
    out = nc.declare_dram_parameter("out", [1, D], F32, isOutput=True)

    with tile.TileContext(nc) as tc:
        with (
            tc.tile_pool(name="const", bufs=1) as const_pool,
            tc.tile_pool(name="hbuf", bufs=1) as hbuf,
            tc.tile_pool(name="vecp", bufs=8) as vec_pool,
            tc.tile_pool(name="pmm", bufs=6, space="PSUM") as psum_mm,
            tc.tile_pool(name="ptr", bufs=2, space="PSUM") as psum_tr,
            tc.tile_pool(name="dram", bufs=1, space="DRAM") as dram_pool,
        ):
            wsb = const_pool.tile([128, 17 * 128], mmdt, name="wsb")
            nc.sync.dma_start(wsb[:, :9 * 128], wmat[:, :9 * 128])
            nc.sync.dma_start(wsb[:, 9 * 128:], wmat[:, 9 * 128:])
            wsb16 = None
            if leaf16:
                wsb16 = const_pool.tile([128, 17 * 128], bf16, name="wsb16")
                nc.sync.dma_start(wsb16[:], wmat16[:])
            ident = const_pool.tile([128, 128], mmdt if mode == "fp16" else F32,
                                    name="ident")
            make_identity(nc, ident)
            # touch Tanh once so Bacc's activation-table load happens during
            # the initial DMA shadow instead of before the first real tanh
            warm_act = const_pool.tile([128, 4], F32, name="warm_act")
            nc.scalar.activation(warm_act[:1, :4], ident[:1, :4],
                                 mybir.ActivationFunctionType.Tanh)

            # persistent transposed h states, one tile per (level, d-half)
            def h_tiles(name, n):
                return [hbuf.tile([128, max(n, 1)], mmdt, name=f"{name}_{kh}", tag=f"{name}_{kh}")
                        for kh in (0, 1)]

            def h_tiles16(name, n):
                return [hbuf.tile([128, max(n, 1)], leafdt, name=f"{name}_{kh}",
                                  tag=f"{name}_{kh}") for kh in (0, 1)]

            hT8 = h_tiles16("hT8", N_LEAF_CORE)
            hT = {7: h_tiles("hT7", 2048), 6: h_tiles("hT6", 512),
                  5: h_tiles("hT5", 128), 4: h_tiles("hT4", 32)}
            # level-3 state and the gathered level-3 array live packed
            # (both d-halves in one tile) so the AG bounce is 1 DMA each way
            NLOC = SIZES[3] // N_CORES                      # 8
            t3p = hbuf.tile([128, 2 * NLOC], mmdt, name="hT3p", tag="hT3p")
            hT[3] = [t3p[:, 0:NLOC], t3p[:, NLOC:2 * NLOC]]
            h3ap = hbuf.tile([128, 2 * SIZES[3]], mmdt, name="h3allp", tag="h3allp")
            h3all = [h3ap[:, 0:SIZES[3]], h3ap[:, SIZES[3]:2 * SIZES[3]]]
            hTt = {3: h_tiles("hTt3", 64), 2: h_tiles("hTt2", 16),
                   1: h_tiles("hTt1", 4), 0: h_tiles("hTt0", 4)}
            hTt1w = h_tiles("hTt1w", 16)

            for rep in range(reps):
                # ---- leaf loader (already transposed on host): 1MB DMAs,
                # interleaved with level-7 chunks so compute hides the stream
                csz = CSZ

                def leaf_chunk(c):
                    if probe == "nodma":
                        if c == 0:
                            for kh in (0, 1):
                                nc.gpsimd.memset(hT8[kh][:], 0.0)
                        return
                    for kh in (0, 1):
                        eng = nc.sync
                        eng.dma_start(
                            hT8[kh][:, c * csz:(c + 1) * csz],
                            leavesT[kh * 128:(kh + 1) * 128, c * csz:(c + 1) * csz])

                def pe_warm(n):
                    for _ in range(n):
                        scr = psum_tr.tile([128, 512], F32, name="ps_tr", tag="tr")
                        nc.tensor.matmul(scr[:, :512], wsb[:, 0:128],
                                         wsb[:, 0:512], start=True, stop=True)

                # preloaded bias arrays: one DMA per tensor per rep, issued
                # late enough not to head-of-line-block the leaf stream
                vloc = vec_pool.tile([128, 2, N_VECS_LOC], mmdt, name="vloc",
                                     tag="vloc", bufs=1)
                vtail = vec_pool.tile([128, 2, N_VECS_TAIL], mmdt, name="vtail",
                                      tag="vtail", bufs=1)

                def load_vloc():
                    nc.sync.dma_start(
                        vloc[:],
                        vecs_locT[:].rearrange("(mh k) n -> k mh n", mh=2))

                def load_vtail():
                    nc.sync.dma_start(
                        vtail[:],
                        vecs_tailT[:].rearrange("(mh k) n -> k mh n", mh=2))

                # ---- shared level routine ----
                def do_level(child, n_par, vec_tile, vec_col0, hT_out,
                             chunk_prologue=None, warm_after=0, wsrc=None,
                             chunk=512, bias_dve=False, rview_override=None):
                    if wsrc is None:
                        wsrc = wsb
                    rview = rview_override or [
                        child[kh][:, :4 * n_par].rearrange(
                            "k (p four) -> k p four", four=4)
                        for kh in (0, 1)]
                    for c0 in range(0, n_par, chunk):
                        if chunk_prologue is not None:
                            chunk_prologue(c0)
                        if warm_after and c0 > 0:
                            pe_warm(warm_after)
                        N = min(chunk, n_par - c0)
                        vts = [vec_tile[:, mh, vec_col0 + c0: vec_col0 + c0 + N]
                               for mh in (0, 1)]
                        for mh in (0, 1):
                            ps = psum_mm.tile([128, 512], F32, name="ps_mm", tag="mm")
                            for i in range(4):
                                for kh in (0, 1):
                                    blk = (9 if mh else 0) + i * 2 + kh
                                    w = wsrc[:, blk * 128:(blk + 1) * 128]
                                    rhs = rview[kh][:, c0:c0 + N, i]
                                    nc.tensor.matmul(ps[:, :N], w, rhs,
                                                     start=(i == 0 and kh == 0),
                                                     stop=False)
                            nc.tensor.matmul(ps[:, :N], wsb[:, 8 * 128:9 * 128],
                                             vts[mh][:, :N], start=False, stop=True)
                            nc.scalar.activation(hT_out[mh][:, c0:c0 + N], ps[:, :N],
                                                 mybir.ActivationFunctionType.Tanh)

                if probe == "A":
                    for c in range(N_LEAF_CORE // csz):
                        leaf_chunk(c)
                    nc.sync.dma_start(out[0:1, 0:16],
                                      hT8[0][:1, :64 // dsz].bitcast(F32))
                    continue

                # ---- local levels 7..4 ----
                col0 = 0
                child = hT8
                for l in LOC_LEVELS:
                    prologue = None
                    if l == 7:
                        chunk_l7 = CHUNKS.get(7, 512) if mode == "fp16" else 512
                        # leaf-DMA chunks consumed per compute chunk
                        R = max(1, 4 * chunk_l7 // csz)
                        ntot = N_LEAF_CORE // csz
                        for c in range(min(2 * R, ntot)):
                            leaf_chunk(c)
                        load_vloc()
                        pe_warm(WARM0)

                        def prologue(c0):
                            ci = c0 // chunk_l7
                            for c in range(R * (ci + 2),
                                           min(R * (ci + 3), ntot)):
                                leaf_chunk(c)
                    do_level(child, LOC_PAR[l], vloc, col0, hT[l],
                             chunk_prologue=prologue,
                             warm_after=WARMC if l == 7 else 0,
                             wsrc=wsb16 if (l == 7 and leaf16) else None,
                             chunk=CHUNKS.get(l, 512) if mode == "fp16" else 512,
                             bias_dve=(l in BIAS_DVE))
                    col0 += LOC_PAR[l]
                    child = hT[l]

                if probe == "B":
                    nc.sync.dma_start(out[0:1, 0:(8 * dsz // 4)], hT[3][0][:1, :8].bitcast(F32))
                    continue

                load_vtail()
                # ---- AllGather of level-3 states, transposed layout ----
                # per-rank bounce [256 d, 8 nodes]; gathered [8*256, 8]
                cc_in = dram_pool.tile([D, NLOC], mmdt, name="cc_in")
                cc_out = dram_pool.tile([N_CORES * D, NLOC], mmdt,
                                        name="cc_out")
                nc.sync.dma_start(
                    cc_in[:].rearrange("(kh k) n -> k kh n", kh=2),
                    t3p[:].rearrange("k (kh n) -> k kh n", kh=2))
                nc.gpsimd.collective_compute(
                    "AllGather", mybir.AluOpType.bypass,
                    replica_groups=[list(range(N_CORES))],
                    ins=[cc_in.opt()], outs=[cc_out.opt()])
                # core r's block lives at rows [256r, 256r+256); fetch the two
                # d-halves of every block into the packed column-concat layout
                cc_v = cc_out[:].rearrange("(r kh k) n -> kh k r n",
                                           r=N_CORES, kh=2)
                for kh in (0, 1):
                    nc.sync.dma_start(
                        h3all[kh].rearrange("k (r n) -> k r n", r=N_CORES),
                        cc_v[kh])

                if probe == "C":
                    nc.sync.dma_start(out[0:1, 0:16], h3all[0][:1, :64 // dsz].bitcast(F32))
                    continue

                # ---- replicated tail levels 2..1 ----
                tail_col0 = {2: 5, 1: 1}
                child = h3all
                for l in (2, 1):
                    do_level(child, SIZES[l], vtail, tail_col0[l], hTt[l])
                    child = hTt[l]
                # root: N=1 matmuls are invalid ISA -> compute 4 replicated
                # roots by re-reading the same children via a step-0
                # broadcast AP (no copies needed)
                root_rv = [hTt[1][kh][:, 0:4].unsqueeze(1)
                           .broadcast_to([128, 4, 4]) for kh in (0, 1)]
                do_level(hTt[1], 4, vtail, int(OFFSETS[4]), hTt[0],
                         rview_override=root_rv)

                # ---- write the root (transpose back to natural) ----
                ps_o = psum_tr.tile([128, 512], mmdt if mode == "fp16" else F32,
                                    name="ps_o", tag="tr")
                for mh in (0, 1):
                    lhs_o = (hTt[0][mh][:, :1] if mode == "fp16"
                             else hTt[0][mh][:, :1].bitcast(F32))
                    nc.tensor.matmul(ps_o[:1, mh * 128:(mh + 1) * 128],
                                     lhs_o, ident[:],
                                     is_transpose=True, start=True, stop=True)
                onat = vec_pool.tile([128, 512], F32, name="onat", tag="vec")
                nc.vector.tensor_copy(onat[:1, :D], ps_o[:1, :D])
                nc.sync.dma_start(out[:], onat[:1, :D])

    nc.finalize()
    _BASS[key] = nc
    return nc


def _prep_inputs(vectors, Wl, Wr, mode="fp16", leaf16=False):
    vectors = np.asarray(vectors, dtype=np.float32)
    Wl = np.asarray(Wl, dtype=np.float32)
    Wr = np.asarray(Wr, dtype=np.float32)

    ind = np.arange(1, B + 1, dtype=np.float32)
    lc = (B - ind) / (B - 1)
    rc = (ind - 1) / (B - 1)
    # W_t[i] = W_i.T laid out [k', (i, kh, mh, m')] for SBUF [128, 2048]
    Wt = np.stack([lc[i] * Wl.T + rc[i] * Wr.T for i in range(B)])  # [4, 256k, 256m]
    # block order: the 8 (i,kh) blocks for mh=0, then identity, then mh=1 —
    # so one contiguous DMA carries everything the first psum group needs
    W5 = Wt.reshape(4, 2, 128, 2, 128)            # [i, kh, k', mh, m']
    halves = [W5[:, :, :, mh, :].reshape(4, 2, 128, 128)
              .transpose(2, 0, 1, 3).reshape(128, 8 * 128) for mh in (0, 1)]
    wmat = np.ascontiguousarray(
        np.concatenate([halves[0], np.eye(128, dtype=np.float32), halves[1]],
                       axis=1), dtype=np.float32)

    # one transposed copy of the node array; all per-core slices are views
    # into it laid out [d, nodes] (part of sharding, no arithmetic)
    vecsT = np.ascontiguousarray(vectors.T)                      # [256, 87381]
    vecs_tailT = np.ascontiguousarray(
        np.concatenate([vecsT[:, :int(OFFSETS[4])],
                        np.repeat(vecsT[:, 0:1], 4, axis=1)], axis=1))
    import ml_dtypes
    hdt = np.float16 if mode == "fp16" else np.float32
    ldt = ml_dtypes.bfloat16 if leaf16 else hdt
    wmat16 = wmat.astype(ml_dtypes.bfloat16)
    in_maps = []
    for c in range(N_CORES):
        o8 = int(OFFSETS[8])
        leavesT_c = vecsT[:, o8 + c * N_LEAF_CORE: o8 + (c + 1) * N_LEAF_CORE]
        loc_parts = []
        for l in LOC_LEVELS:
            npl = LOC_PAR[l]
            o = int(OFFSETS[l])
            loc_parts.append(vecsT[:, o + c * npl: o + (c + 1) * npl])
        im = {
            "leavesT": np.ascontiguousarray(leavesT_c).astype(ldt),
            "vecs_locT": np.ascontiguousarray(
                np.concatenate(loc_parts, axis=1)).astype(hdt),
            "vecs_tailT": vecs_tailT.astype(hdt),
            "wmat": wmat.astype(hdt),
        }
        if leaf16:
            im["wmat16"] = wmat16
        in_maps.append(im)
    return in_maps


def kernel(vectors, Wl, Wr, branching, n_levels, _mode="fp16"):
    _lazy_imports()
    assert int(branching) == B and int(n_levels) == L
    vectors = np.asarray(vectors)
    assert vectors.shape == (int(OFFSETS[L]), D), vectors.shape

    nc = _build_nc(mode=_mode)
    in_maps = _prep_inputs(vectors, Wl, Wr, mode=_mode)
    try:
        res = run_bass_kernel_spmd(nc, in_maps, core_ids=list(range(N_CORES)),
                                   trace=False)
    except Exception:
        # transient device hiccups (e.g. NRT_EXEC_UNIT_UNRECOVERABLE right
        # after another process released the cores) clear on a retry
        res = run_bass_kernel_spmd(nc, in_maps, core_ids=list(range(N_CORES)),
                                   trace=False)
    root = res.results[0]["out"]
    return np.asarray(root, dtype=np.float32).reshape(1, D)



# revision 9
# speedup vs baseline: 1.0782x; 1.0782x over previous
"""CBTree bottom-up fold kernel for 8 trn2 NeuronCores.

Problem: complete 4-ary tree, 9 levels, 87381 nodes in BFS order, d=256.
  leaves (level 8): h = vectors[21845:]
  internal node:    h = tanh(sum_i W_i @ h_child_i + vectors[node])
  where W_i = lc[i]*Wl + rc[i]*Wr,  lc=[1,2/3,1/3,0], rc=[0,1/3,2/3,1].

Strategy (data-parallel over sibling groups):
  - Shard every level contiguously over 8 cores. Children of a core's
    parents are exactly the core's own previous-level outputs, so
    levels 7..2 run with zero communication (level-2 node p's children
    are level-3 nodes 4p..4p+3, all owned by core p//2).
  - One 8KB AllGather of the level-2 states (16 nodes), then every
    core redundantly folds level 1 and the root.
  - On chip h lives transposed ([d, nodes], d split into two 128-row
    partition halves) so the tensor engine contracts over d; the
    host-side sharding step hands each core its slices already in this
    layout. Level l is 8 accumulating 128x128xN matmuls per output
    half (4 sibling positions x 2 d-halves), a 9th identity-weight
    matmul that adds the bias vector in PSUM, and a tanh on the scalar
    engine writing the next level's tile.
  - fp16 streams (fp32 PSUM accumulation): ~3.4e-3 scale-relative
    error vs the fp32 reference.
  - The leaf DMA stream is interleaved with the per-level bias-vector
    loads chunk by chunk so no large transfer head-of-line-blocks the
    stream, and the PE is kept continuously busy from ~0.8us with
    warm-up matmuls so the p-state ramp (full clock only after 3us of
    uninterrupted work) completes before the first real matmul.
  - The root would be an N=1 matmul (invalid ISA), so the last level
    computes 4 replicated copies of the root; the root state is written
    back with a single partition-strided DMA (no transpose matmul).
"""

import numpy as np

F32 = None  # set on first _lazy_imports()

_BASS = {}


def _lazy_imports():
    global bass, bacc, mybir, tile, make_identity, run_bass_kernel_spmd, F32
    import concourse.bass as bass
    import concourse.mybir as mybir
    from concourse import bacc
    import concourse.tile as tile
    from concourse.masks import make_identity
    from concourse.bass_utils import run_bass_kernel_spmd
    F32 = mybir.dt.float32


N_CORES = 8
D = 256
B = 4
L = 9
SIZES = [B**l for l in range(L)]            # [1,4,16,64,256,1024,4096,16384,65536]
OFFSETS = np.concatenate([[0], np.cumsum(SIZES)])  # [0,1,5,21,85,341,1365,5461,21845,87381]
N_LEAF_CORE = SIZES[8] // N_CORES           # 8192
# local (sharded) levels produce parents at levels 7..2
LOC_LEVELS = [7, 6, 5, 4, 3, 2]
LOC_PAR = {l: SIZES[l] // N_CORES for l in LOC_LEVELS}   # 2048,512,128,32,8,2
N_VECS_LOC = sum(LOC_PAR.values())          # 2730
N_VECS_TAIL = 8                             # 4x replicated root + 4 level-1 rows


def _build_nc(mode="fp16", WARM0=9, CHUNKS=None, CSZ=1024):
    if CHUNKS is None:
        CHUNKS = {7: 256, 6: 128, 5: 64, 4: 16}
    key = ("nc", mode, WARM0, tuple(sorted(CHUNKS.items())), CSZ)
    if key in _BASS:
        return _BASS[key]
    nc = bacc.Bacc(num_devices=N_CORES)
    mmdt = {"f32r": mybir.dt.float32r, "fp32": F32,
            "fp16": mybir.dt.float16}[mode]

    # all h/vec tensors arrive transposed: [256 = 2x128 d-rows, n nodes]
    leavesT = nc.declare_dram_parameter("leavesT", [D, N_LEAF_CORE], mmdt, isOutput=False)
    vecs_locT = nc.declare_dram_parameter("vecs_locT", [D, N_VECS_LOC], mmdt, isOutput=False)
    vecs_tailT = nc.declare_dram_parameter("vecs_tailT", [D, N_VECS_TAIL], mmdt, isOutput=False)
    wmat = nc.declare_dram_parameter("wmat", [128, 17 * 128], mmdt, isOutput=False)
    out = nc.declare_dram_parameter("out", [1, D], F32, isOutput=True)

    with tile.TileContext(nc) as tc:
        with (
            tc.tile_pool(name="const", bufs=1) as const_pool,
            tc.tile_pool(name="hbuf", bufs=1) as hbuf,
            tc.tile_pool(name="vecp", bufs=8) as vec_pool,
            tc.tile_pool(name="pmm", bufs=6, space="PSUM") as psum_mm,
            tc.tile_pool(name="ptr", bufs=2, space="PSUM") as psum_tr,
            tc.tile_pool(name="dram", bufs=1, space="DRAM") as dram_pool,
        ):
            wsb = const_pool.tile([128, 17 * 128], mmdt, name="wsb")
            # warm-up source: zeroed quickly by the Pool engine so the PE can
            # start clocking up before any DMA lands
            warm_src = const_pool.tile([128, 512], mmdt, name="warm_src")
            nc.gpsimd.memset(warm_src[:], 0.0)

            def pe_warm(n):
                for _ in range(n):
                    scr = psum_tr.tile([128, 512], F32, name="ps_tr", tag="tr")
                    nc.tensor.matmul(scr[:, :512], warm_src[:, 0:128],
                                     warm_src[:, :512], start=True, stop=True)

            pe_warm(WARM0)
            nc.sync.dma_start(wsb[:, :9 * 128], wmat[:, :9 * 128])
            nc.sync.dma_start(wsb[:, 9 * 128:], wmat[:, 9 * 128:])
            ident = const_pool.tile([128, 128], mmdt if mode == "fp16" else F32,
                                    name="ident")
            make_identity(nc, ident)
            # touch Tanh once so Bacc's activation-table load happens during
            # the initial DMA shadow instead of before the first real tanh
            warm_act = const_pool.tile([128, 4], F32, name="warm_act")
            nc.scalar.activation(warm_act[:1, :4], ident[:1, :4],
                                 mybir.ActivationFunctionType.Tanh)

            # persistent transposed h states, one tile per (level, d-half)
            def h_tiles(name, n):
                return [hbuf.tile([128, max(n, 1)], mmdt, name=f"{name}_{kh}", tag=f"{name}_{kh}")
                        for kh in (0, 1)]

            hT8 = h_tiles("hT8", N_LEAF_CORE)
            hT = {7: h_tiles("hT7", 2048), 6: h_tiles("hT6", 512),
                  5: h_tiles("hT5", 128), 4: h_tiles("hT4", 32),
                  3: h_tiles("hT3", 8)}
            # level-2 state and the gathered level-2 array live packed
            # (both d-halves in one tile) so the AG bounce is 1 DMA in
            NLOC = LOC_PAR[2]                              # 2
            t2p = hbuf.tile([128, 2 * NLOC], mmdt, name="hT2p", tag="hT2p")
            hT[2] = [t2p[:, 0:NLOC], t2p[:, NLOC:2 * NLOC]]
            h2ap = hbuf.tile([128, 2 * SIZES[2]], mmdt, name="h2allp",
                             tag="h2allp")
            h2all = [h2ap[:, 0:SIZES[2]], h2ap[:, SIZES[2]:2 * SIZES[2]]]
            hTt1 = h_tiles("hTt1", 4)
            # root states in fp32 so the final DMA needs no conversion
            t0 = hbuf.tile([128, 2, 4], F32, name="hTt0", tag="hTt0")
            hTt0 = [t0[:, 0, :], t0[:, 1, :]]

            # ---- leaf loader (already transposed on host): 512KB chunks,
            # interleaved with the per-chunk level-7 bias columns
            csz = CSZ
            ntot = N_LEAF_CORE // csz

            def leaf_chunk(c):
                for kh in (0, 1):
                    nc.sync.dma_start(
                        hT8[kh][:, c * csz:(c + 1) * csz],
                        leavesT[kh * 128:(kh + 1) * 128, c * csz:(c + 1) * csz])

            vloc = vec_pool.tile([128, 2, N_VECS_LOC], mmdt, name="vloc",
                                 tag="vloc", bufs=1)
            vtail = vec_pool.tile([128, 2, N_VECS_TAIL], mmdt, name="vtail",
                                  tag="vtail", bufs=1)
            vloc_src = vecs_locT[:].rearrange("(mh k) n -> k mh n", mh=2)

            def load_vloc_cols(c0, c1):
                nc.sync.dma_start(vloc[:, :, c0:c1], vloc_src[:, :, c0:c1])

            def load_vtail():
                nc.sync.dma_start(
                    vtail[:],
                    vecs_tailT[:].rearrange("(mh k) n -> k mh n", mh=2))

            # ---- shared level routine ----
            def do_level(child, n_par, vec_tile, vec_col0, hT_out,
                         chunk_prologue=None, chunk=512, rview_override=None,
                         out_f32=False):
                rview = rview_override or [
                    child[kh][:, :4 * n_par].rearrange(
                        "k (p four) -> k p four", four=4)
                    for kh in (0, 1)]
                for c0 in range(0, n_par, chunk):
                    if chunk_prologue is not None:
                        chunk_prologue(c0)
                    N = min(chunk, n_par - c0)
                    vts = [vec_tile[:, mh, vec_col0 + c0: vec_col0 + c0 + N]
                           for mh in (0, 1)]
                    for mh in (0, 1):
                        ps = psum_mm.tile([128, 512], F32, name="ps_mm", tag="mm")
                        for i in range(4):
                            for kh in (0, 1):
                                blk = (9 if mh else 0) + i * 2 + kh
                                w = wsb[:, blk * 128:(blk + 1) * 128]
                                rhs = rview[kh][:, c0:c0 + N, i]
                                nc.tensor.matmul(ps[:, :N], w, rhs,
                                                 start=(i == 0 and kh == 0),
                                                 stop=False)
                        nc.tensor.matmul(ps[:, :N], wsb[:, 8 * 128:9 * 128],
                                         vts[mh][:, :N], start=False, stop=True)
                        nc.scalar.activation(hT_out[mh][:, c0:c0 + N], ps[:, :N],
                                             mybir.ActivationFunctionType.Tanh)

            # ---- local levels 7..2 ----
            col0 = 0
            child = hT8
            for l in LOC_LEVELS:
                prologue = None
                if l == 7:
                    chunk_l7 = CHUNKS.get(7, 512) if mode == "fp16" else 512
                    # leaf-DMA chunks consumed per compute chunk
                    R = max(1, 4 * chunk_l7 // csz)
                    nck = LOC_PAR[7] // chunk_l7
                    for c in range(min(2 * R, ntot)):
                        leaf_chunk(c)
                    for ci in range(min(2, nck)):
                        load_vloc_cols(ci * chunk_l7, (ci + 1) * chunk_l7)

                    def prologue(c0):
                        ci = c0 // chunk_l7
                        for c in range(R * (ci + 2),
                                       min(R * (ci + 3), ntot)):
                            leaf_chunk(c)
                        if ci + 2 < nck:
                            load_vloc_cols((ci + 2) * chunk_l7,
                                           (ci + 3) * chunk_l7)
                        if ci == nck - 2:
                            # rest of the bias columns (levels 6..2) + tail
                            load_vloc_cols(LOC_PAR[7], N_VECS_LOC)
                            load_vtail()
                do_level(child, LOC_PAR[l], vloc, col0, hT[l],
                         chunk_prologue=prologue,
                         chunk=CHUNKS.get(l, 512) if mode == "fp16" else 512)
                col0 += LOC_PAR[l]
                child = hT[l]

            # ---- AllGather of level-2 states, transposed layout ----
            # per-rank bounce [256 d, 2 nodes]; gathered [8*256, 2]
            cc_in = dram_pool.tile([D, NLOC], mmdt, name="cc_in")
            cc_out = dram_pool.tile([N_CORES * D, NLOC], mmdt,
                                    name="cc_out")
            nc.sync.dma_start(
                cc_in[:].rearrange("(kh k) n -> k kh n", kh=2),
                t2p[:].rearrange("k (kh n) -> k kh n", kh=2))
            nc.gpsimd.collective_compute(
                "AllGather", mybir.AluOpType.bypass,
                replica_groups=[list(range(N_CORES))],
                ins=[cc_in.opt()], outs=[cc_out.opt()])
            # core r's block lives at rows [256r, 256r+256); fetch the two
            # d-halves of every block into the packed column-concat layout
            cc_v = cc_out[:].rearrange("(r kh k) n -> kh k r n",
                                       r=N_CORES, kh=2)
            for kh in (0, 1):
                nc.sync.dma_start(
                    h2all[kh].rearrange("k (r n) -> k r n", r=N_CORES),
                    cc_v[kh])

            # ---- replicated tail: level 1, then 4 copies of the root ----
            do_level(h2all, SIZES[1], vtail, 4, hTt1)
            root_rv = [hTt1[kh][:, 0:4].unsqueeze(1)
                       .broadcast_to([128, 4, 4]) for kh in (0, 1)]
            do_level(hTt1, 4, vtail, 0, hTt0, rview_override=root_rv)

            # ---- write the root: partition-strided DMA, no transpose ----
            nc.sync.dma_start(
                out[0:1, :].rearrange("o (kh k) -> k kh o", kh=2),
                t0[:, :, 0:1])

    nc.finalize()
    _BASS[key] = nc
    return nc


def _prep_inputs(vectors, Wl, Wr, mode="fp16"):
    vectors = np.asarray(vectors, dtype=np.float32)
    Wl = np.asarray(Wl, dtype=np.float32)
    Wr = np.asarray(Wr, dtype=np.float32)

    ind = np.arange(1, B + 1, dtype=np.float32)
    lc = (B - ind) / (B - 1)
    rc = (ind - 1) / (B - 1)
    # W_t[i] = W_i.T laid out [k', (i, kh, mh, m')] for SBUF [128, 2048]
    Wt = np.stack([lc[i] * Wl.T + rc[i] * Wr.T for i in range(B)])  # [4, 256k, 256m]
    # block order: the 8 (i,kh) blocks for mh=0, then identity, then mh=1 —
    # so one contiguous DMA carries everything the first psum group needs
    W5 = Wt.reshape(4, 2, 128, 2, 128)            # [i, kh, k', mh, m']
    halves = [W5[:, :, :, mh, :].reshape(4, 2, 128, 128)
              .transpose(2, 0, 1, 3).reshape(128, 8 * 128) for mh in (0, 1)]
    wmat = np.ascontiguousarray(
        np.concatenate([halves[0], np.eye(128, dtype=np.float32), halves[1]],
                       axis=1), dtype=np.float32)

    # one transposed copy of the node array; all per-core slices are views
    # into it laid out [d, nodes] (part of sharding, no arithmetic)
    vecsT = np.ascontiguousarray(vectors.T)                      # [256, 87381]
    # tail biases: 4 replicated root rows, then the 4 level-1 rows
    vecs_tailT = np.ascontiguousarray(
        np.concatenate([np.repeat(vecsT[:, 0:1], 4, axis=1),
                        vecsT[:, 1:5]], axis=1))
    hdt = np.float16 if mode == "fp16" else np.float32
    in_maps = []
    for c in range(N_CORES):
        o8 = int(OFFSETS[8])
        leavesT_c = vecsT[:, o8 + c * N_LEAF_CORE: o8 + (c + 1) * N_LEAF_CORE]
        loc_parts = []
        for l in LOC_LEVELS:
            npl = LOC_PAR[l]
            o = int(OFFSETS[l])
            loc_parts.append(vecsT[:, o + c * npl: o + (c + 1) * npl])
        im = {
            "leavesT": np.ascontiguousarray(leavesT_c).astype(hdt),
            "vecs_locT": np.ascontiguousarray(
                np.concatenate(loc_parts, axis=1)).astype(hdt),
            "vecs_tailT": vecs_tailT.astype(hdt),
            "wmat": wmat.astype(hdt),
        }
        in_maps.append(im)
    return in_maps


def kernel(vectors, Wl, Wr, branching, n_levels, _mode="fp16"):
    _lazy_imports()
    assert int(branching) == B and int(n_levels) == L
    vectors = np.asarray(vectors)
    assert vectors.shape == (int(OFFSETS[L]), D), vectors.shape

    nc = _build_nc(mode=_mode)
    in_maps = _prep_inputs(vectors, Wl, Wr, mode=_mode)
    try:
        res = run_bass_kernel_spmd(nc, in_maps, core_ids=list(range(N_CORES)),
                                   trace=False)
    except Exception:
        # transient device hiccups (e.g. NRT_EXEC_UNIT_UNRECOVERABLE right
        # after another process released the cores) clear on a retry
        res = run_bass_kernel_spmd(nc, in_maps, core_ids=list(range(N_CORES)),
                                   trace=False)
    root = res.results[0]["out"]
    return np.asarray(root, dtype=np.float32).reshape(1, D)
